# revision 45
# baseline (speedup 1.0000x reference)
"""CFBlock (GNN message passing) Trainium2 Bass kernel.

Sharding: edges sorted by dst; each of the 8 cores owns a contiguous range of
1250 destination nodes and all edges pointing into it. Each core:
  - (replicated) computes h_pre = LN1(x) @ W_pre + b_pre for ALL nodes and
    stores it as a bf16 table in DRAM,
  - gathers h_pre[src] for its edges with dma_gather, computes the edge filter
    GEMM, multiplies, and segment-sums via one-hot matmuls into PSUM windows
    of 128 destination nodes,
  - runs post-Linear + SiLU + residual + LN2 + FFN + residual for its nodes.
No collectives; the host concatenates the 8 output slices.

V2 notes (post-trace):
  - bulk DMAs (rbT, h_pre, out) ride HWDGE (nc.sync) so GpSimd is gather-only
  - one-hot built with 2-op tensor_scalar (subtract, is_equal) -- the 1-op
    is_equal encoding measured ~1us on HW
  - filter GEMM runs 2 blocks per PSUM bank; PSUM->SBUF casts ride nc.any
    (scheduler balances ACT/DVE); edge multiply is bf16 TT at 2x
  - gathers skip trailing pad rows (idx -1) with memset-zeroed tails
  - zero biases (the common case) skip their add instructions entirely;
    SiLU runs straight from PSUM on the scalar engine with fused bias
"""

import sys
import types

import numpy as np
import ml_dtypes

import concourse.bass as bass
import concourse.mybir as mybir
from concourse import bacc
from concourse import library_config
from concourse.tile import TileContext
from concourse import bass_utils

# Defensive: if BASS_TRACE is set but the image's antenv lacks axon_hooks,
# run_bass_kernel_spmd would crash on import. Provide a null hook module so
# bass_utils falls back to running untraced.
if "antenv.axon_hooks" not in sys.modules:
    try:
        import antenv

        _m = types.ModuleType("antenv.axon_hooks")
        _m._hook = None
        _m.set_axon_ntff_profile_hook = lambda h: setattr(_m, "_hook", h)
        _m.get_axon_ntff_profile_hook = lambda: _m._hook
        sys.modules["antenv.axon_hooks"] = _m
        antenv.axon_hooks = _m
    except ImportError:
        pass

BF16 = ml_dtypes.bfloat16

N_NODES = 10000
N_EDGES = 320000
D = 256          # d_model
DR = 128         # d_radial
DH = 256         # d_hidden
DFF = 1024
EPS = 1e-5
NCORES = 8
NPC = 1250       # nodes per core
NWIN = 10        # 128-node windows per core (last window: 98 valid nodes)
NPAD = 10112     # 79 * 128
NT = NPAD // 128  # 79 node tiles
XCH = 16         # node tiles per xT DMA chunk
RBCH = 16        # edge blocks per rbT DMA chunk
HB = 16          # h_pre tiles batched per DMA write

AF = mybir.ActivationFunctionType
OP = mybir.AluOpType

# packed f32 const columns: bpre, bpost, bff2, iota, bff1, eps, dstloc[, bfilt]
C_BPRE, C_BPOST, C_BFF2 = 0, 256, 512
C_IOTA, C_BFF1, C_EPS, C_DSTLOC = 768, 896, 904, 905
# packed bf16 const columns
W_PRE, W_FILT, W_POST, W_FF1, W_FF2, W_ID = 0, 512, 768, 1280, 3328, 5376
W_TOT = 5504


def _f32(a):
    return np.ascontiguousarray(a, dtype=np.float32)


def _bf(a):
    return np.ascontiguousarray(np.asarray(a, dtype=np.float32).astype(BF16))


def _build_program(geo: dict, flags: dict, phase: int = 4):
    nc = bacc.Bacc("TRN2", target_bir_lowering=False, debug=False)
    dt = mybir.dt

    has_bfilt = flags["bfilt"]
    has_bpre = flags["bpre"]
    has_bpost = flags["bpost"]
    has_bff2 = flags["bff2"]

    BPe, BPo, BS = geo["BPe"], geo["BPo"], geo["BS"]
    mp = geo["mp"]            # [NWIN][3] first block to memset per region
    Bw = 2 * BPe + 2 * BPo + BS   # edge blocks per window
    NIW = (BPe + BPo + BS) * 128  # gather indices per window
    EPW = Bw * 128            # padded edges per window
    NBLK = NWIN * Bw          # padded blocks per core
    CW = C_EPS + 1 + (DH if has_bfilt else 0)
    C_BFILT = C_EPS + 1

    # ---- I/O ----
    xnm_d = nc.dram_tensor("xnm", [NPAD, D], dt.bfloat16, kind="ExternalInput")
    xres_d = nc.dram_tensor("xres", [NWIN * 128, D], dt.float32, kind="ExternalInput")
    rbT_d = nc.dram_tensor("rbT", [DR, NBLK * 128], dt.bfloat16, kind="ExternalInput")
    ohp_d = nc.dram_tensor("ohp", [128, NBLK * 128], dt.bfloat16, kind="ExternalInput")
    gidx_d = nc.dram_tensor("gidx", [NWIN, 128, NIW // 16], dt.int16, kind="ExternalInput")
    cpack_d = nc.dram_tensor("cpack", [128, CW], dt.float32, kind="ExternalInput")
    wpack_d = nc.dram_tensor("wpack", [128, W_TOT], dt.bfloat16, kind="ExternalInput")
    out_d = nc.dram_tensor("out", [NWIN * 128, D], dt.float32, kind="ExternalOutput")

    with TileContext(nc) as tc:
        with (
            tc.tile_pool(name="consts", bufs=1) as consts,
            tc.tile_pool(name="dram", bufs=1, space="DRAM") as dramp,
            tc.tile_pool(name="n1", bufs=12) as n1p,
            tc.tile_pool(name="hout", bufs=2) as houtp,
            tc.tile_pool(name="rbt", bufs=3) as rbtp,
            tc.tile_pool(name="ohp", bufs=3) as ohpp,
            tc.tile_pool(name="gp", bufs=3) as gpp,
            tc.tile_pool(name="gs", bufs=3) as gsp,
            tc.tile_pool(name="edge", bufs=10) as edgep,
            # PSUM budget is 8 banks of 2KB/partition, sized to exactly 8:
            # fps(2) + hagg/pps(2) + tr(1) + mm256(1) + f1ps(2|1) + gagg(0|1)
            tc.tile_pool(name="fps", bufs=2, space="PSUM") as fpsp,
            tc.tile_pool(name="hagg", bufs=2, space="PSUM") as haggp,
            tc.tile_pool(name="n2", bufs=2) as n2p,
            tc.tile_pool(name="n2ps1", bufs=1, space="PSUM") as n2ps1,
            tc.tile_pool(name="n2ps2", bufs=1 if has_bfilt else 2,
                         space="PSUM") as n2ps2,
            tc.tile_pool(name="gaggp", bufs=1, space="PSUM") as gaggp,
        ):
            nc.gpsimd.load_library(library_config.mlp)
            cpk = consts.tile([128, CW], dt.float32, tag="cpack")
            nc.sync.dma_start(out=cpk[:], in_=cpack_d[:])
            wpk = consts.tile([128, W_TOT], dt.bfloat16, tag="wpack")
            nc.sync.dma_start(out=wpk[:], in_=wpack_d[:])
            gidx_sb = consts.tile([128, NWIN, NIW // 16], dt.int16, tag="gidx")
            nc.sync.dma_start(out=gidx_sb[:],
                              in_=gidx_d[:].rearrange("w p s -> p w s"))
            xnm_r = xnm_d[:].rearrange("(t p) n -> t p n", p=128)
            xnm_sb = consts.tile([128, NT, D], dt.bfloat16, tag="xbig")
            nc.sync.dma_start(out=xnm_sb[:],
                              in_=xnm_r.rearrange("t p n -> p t n"))
            xres_r = xres_d[:].rearrange("(w p) n -> w p n", p=128)
            xres_sb = consts.tile([128, NWIN, D], dt.float32, tag="xresb")
            nc.sync.dma_start(out=xres_sb[:],
                              in_=xres_r.rearrange("w p n -> p w n"))
            stds = consts.tile([128, NT + NWIN], dt.float32, tag="stds")

            bpre_sb = cpk[:, C_BPRE:C_BPRE + DH]
            bpost_sb = cpk[:, C_BPOST:C_BPOST + D]
            bff2_sb = cpk[:, C_BFF2:C_BFF2 + D]

            bff1_sb = cpk[:, C_BFF1:C_BFF1 + 8]
            eps_sb = cpk[:, C_EPS:C_EPS + 1]

            bfilt_sb = cpk[:, C_BFILT:C_BFILT + DH] if has_bfilt else None
            wpre_k = lambda k: wpk[:, W_PRE + k * DH:W_PRE + (k + 1) * DH]
            wfilt_sb = wpk[:, W_FILT:W_FILT + DH]
            wpost_k = lambda k: wpk[:, W_POST + k * D:W_POST + (k + 1) * D]
            wff1_k = lambda k: wpk[:, W_FF1 + k * DFF:W_FF1 + (k + 1) * DFF]
            wff2_s = lambda s: wpk[:, W_FF2 + s * D:W_FF2 + (s + 1) * D]
            ident_sb = wpk[:, W_ID:W_ID + 128]

            hpre_dram = dramp.tile([NPAD, D], dt.bfloat16, tag="hpre")
            hpre_r = hpre_dram[:].rearrange("(t p) n -> t p n", p=128)

            # ---- node phase 1: h_pre for all nodes ----
            h_big = None
            for t in range(NT):
                if t % HB == 0:
                    h_big = houtp.tile([128, HB, DH], dt.bfloat16, tag="hsb")
                x_sb = xnm_sb[:, t, :]
                stats = n1p.tile([128, 6], dt.float32, tag="bnst")
                nc.vector.bn_stats(out=stats[:], in_=x_sb)
                mv = n1p.tile([128, 2], dt.float32, tag="bnagg")
                nc.vector.bn_aggr(out=mv[:], in_=stats[:])
                nc.scalar.activation(stds[:, t:t + 1], mv[:, 1:2], AF.Sqrt,
                                     bias=eps_sb)
                rstd = n1p.tile([128, 1], dt.float32, tag="rstd")
                nc.vector.reciprocal(out=rstd[:], in_=stds[:, t:t + 1])
                z = n1p.tile([128, D], dt.bfloat16, tag="z")
                nc.vector.tensor_scalar(out=z[:], in0=x_sb,
                                        scalar1=mv[:, 0:1], scalar2=rstd[:],
                                        op0=OP.subtract, op1=OP.mult)
                ztps = n2ps1.tile([128, 2, 128], dt.bfloat16, tag="tr")
                nc.tensor.transpose(ztps[:, 0, :], z[:, 0:128], ident_sb)
                nc.tensor.transpose(ztps[:, 1, :], z[:, 128:256], ident_sb)
                zT = n1p.tile([128, 2, 128], dt.bfloat16, tag="zT")
                nc.any.tensor_copy(out=zT[:, 0, :], in_=ztps[:, 0, :])
                nc.any.tensor_copy(out=zT[:, 1, :], in_=ztps[:, 1, :])
                pps = haggp.tile([128, DH], dt.float32, tag="hagg")
                nc.tensor.matmul(pps[:], lhsT=zT[:, 0, :],
                                 rhs=wpre_k(0), start=True, stop=False)
                nc.tensor.matmul(pps[:], lhsT=zT[:, 1, :],
                                 rhs=wpre_k(1), start=False, stop=True)
                if has_bpre:
                    nc.vector.tensor_tensor(out=h_big[:, t % HB, :], in0=pps[:],
                                            in1=bpre_sb, op=OP.add)
                else:
                    nc.any.tensor_copy(out=h_big[:, t % HB, :], in_=pps[:])
                if t % HB == HB - 1 or t == NT - 1:
                    t0 = (t // HB) * HB
                    nb = t - t0 + 1
                    nc.sync.dma_start(
                        out=hpre_r[t0:t0 + nb].rearrange("t p n -> p t n"),
                        in_=h_big[:, :nb, :])

            # ---- edge phase + per-window epilogue ----
            out_r = out_d[:].rearrange("(w p) n -> w p n", p=128)
            # gather source views: pair rows fetch 2 consecutive h_pre rows
            hpe_v = hpre_dram[0:NPAD, :].rearrange("(a two) n -> a (two n)",
                                                   two=2)
            hpo_v = hpre_dram[1:NPAD - 1, :].rearrange(
                "(a two) n -> a (two n)", two=2)
            regions = [(BPe, 512, hpe_v), (BPo, 512, hpo_v),
                       (BS, 256, hpre_dram[:])]
            rbt_chunks = {}
            oh_chunks = {}

            def get_rbt(j):
                k = j // RBCH
                if k not in rbt_chunks:
                    c0 = k * RBCH * 128
                    ncols = min(RBCH * 128, NBLK * 128 - c0)
                    t = rbtp.tile([128, RBCH * 128], dt.bfloat16, tag="rbt")
                    nc.sync.dma_start(out=t[:, :ncols],
                                      in_=rbT_d[:, c0:c0 + ncols])
                    rbt_chunks[k] = t
                return rbt_chunks[k]

            def get_oh(j):
                k = j // RBCH
                if k not in oh_chunks:
                    c0 = k * RBCH * 128
                    ncols = min(RBCH * 128, NBLK * 128 - c0)
                    t = ohpp.tile([128, RBCH * 128], dt.bfloat16, tag="ohp")
                    nc.sync.dma_start(out=t[:, :ncols],
                                      in_=ohp_d[:, c0:c0 + ncols])
                    oh_chunks[k] = t
                return oh_chunks[k]
            for w in range(NWIN if phase >= 2 else 0):
                g_views = []
                offc = 0  # column offset into gidx_sb (16-wrapped idx units)
                for ri, (RB, esz, src_view) in enumerate(regions):
                    pool = gsp if ri == 2 else gpp
                    gt = pool.tile([128, RB, esz], dt.bfloat16, tag=f"g{ri}")
                    nc.gpsimd.dma_gather(
                        gt[:], src_view,
                        gidx_sb[:, w, offc:offc + RB * 8],
                        RB * 128, RB * 128, esz, single_packet=False)
                    offc += RB * 8
                    nb_view = RB * esz // DH
                    g_views.append(
                        (gt[:].rearrange("p b (k n) -> p (b k) n", n=DH),
                         nb_view))
                if phase < 3:
                    continue
                hagg = haggp.tile([128, DH], dt.float32, tag="hagg")
                gagg = None
                if has_bfilt:
                    gagg = gaggp.tile([128, DH], dt.float32, tag="gagg")
                jw = w * Bw  # running global block index
                for g_view, nbv in g_views:
                    for p in range((nbv + 1) // 2):
                        b0g = 2 * p
                        nb = min(2, nbv - b0g)
                        j0 = jw + b0g
                        # filter GEMM for up to two blocks into one PSUM bank
                        fps = fpsp.tile([128, 2, DH], dt.float32, tag="fps")
                        for i in range(nb):
                            rbt_sb = get_rbt(j0 + i)
                            boff = ((j0 + i) % RBCH) * 128
                            nc.tensor.matmul(fps[:, i, :],
                                             lhsT=rbt_sb[:, boff:boff + 128],
                                             rhs=wfilt_sb, start=True, stop=True)
                        fsb = edgep.tile([128, 2, DH], dt.bfloat16, tag="fsb")
                        nc.any.tensor_copy(out=fsb[:, :nb, :], in_=fps[:, :nb, :])
                        m_sb = edgep.tile([128, 2, DH], dt.bfloat16, tag="m")
                        nc.vector.tensor_tensor(
                            out=m_sb[:, :nb, :], in0=fsb[:, :nb, :],
                            in1=g_view[:, b0g:b0g + nb, :], op=OP.mult)
                        for i in range(nb):
                            j = j0 + i
                            oh_sb = get_oh(j)
                            boff = (j % RBCH) * 128
                            oh = oh_sb[:, boff:boff + 128]
                            nc.tensor.matmul(hagg[:], lhsT=oh,
                                             rhs=m_sb[:, i, :],
                                             start=(j == w * Bw),
                                             stop=(j == w * Bw + Bw - 1))
                            if has_bfilt:
                                nc.tensor.matmul(gagg[:], lhsT=oh,
                                                 rhs=g_view[:, b0g + i, :],
                                                 start=(j == w * Bw),
                                                 stop=(j == w * Bw + Bw - 1))
                    jw += nbv

                if phase < 4:
                    continue
                # ---- epilogue for this window ----
                outb = n2p.tile([128, D], dt.float32, tag="outb")
                f1sil = n2p.tile([128, 8, 128], dt.bfloat16, tag="f1sil")
                hagg_sb = n2p.tile([128, DH], dt.bfloat16, tag="haggsb")
                if has_bfilt:
                    tmpb = n2p.tile([128, DH], dt.float32, tag="tmpb")
                    nc.vector.tensor_tensor(out=tmpb[:], in0=gagg[:],
                                            in1=bfilt_sb, op=OP.mult)
                    nc.vector.tensor_tensor(out=hagg_sb[:], in0=hagg[:],
                                            in1=tmpb[:], op=OP.add)
                else:
                    nc.any.tensor_copy(out=hagg_sb[:], in_=hagg[:])
                tps = n2ps1.tile([128, 2, 128], dt.bfloat16, tag="tr")
                nc.tensor.transpose(tps[:, 0, :], hagg_sb[:, 0:128], ident_sb)
                nc.tensor.transpose(tps[:, 1, :], hagg_sb[:, 128:256], ident_sb)
                haggT = n2p.tile([128, 2, 128], dt.bfloat16, tag="haggT")
                nc.any.tensor_copy(out=haggT[:, 0, :], in_=tps[:, 0, :])
                nc.any.tensor_copy(out=haggT[:, 1, :], in_=tps[:, 1, :])
                pops = n2ps1.tile([128, D], dt.float32, tag="mm256")
                nc.tensor.matmul(pops[:], lhsT=haggT[:, 0, :],
                                 rhs=wpost_k(0), start=True, stop=False)
                nc.tensor.matmul(pops[:], lhsT=haggT[:, 1, :],
                                 rhs=wpost_k(1), start=False, stop=True)
                if has_bpost:
                    ps_sb = n2p.tile([128, D], dt.float32, tag="pssb")
                    nc.vector.tensor_tensor(out=ps_sb[:], in0=pops[:],
                                            in1=bpost_sb, op=OP.add)
                    nc.scalar.activation(outb[:], ps_sb[:], AF.Silu)
                else:
                    nc.scalar.activation(outb[:], pops[:], AF.Silu)
                x1 = n2p.tile([128, D], dt.float32, tag="x1")
                nc.vector.tensor_tensor(out=x1[:], in0=outb[:],
                                        in1=xres_sb[:, w, :], op=OP.add)
                # LN2
                st2 = n1p.tile([128, 6], dt.float32, tag="bnst")
                nc.vector.bn_stats(out=st2[:], in_=x1[:])
                mv2 = n1p.tile([128, 2], dt.float32, tag="bnagg")
                nc.vector.bn_aggr(out=mv2[:], in_=st2[:])
                nc.scalar.activation(stds[:, NT + w:NT + w + 1], mv2[:, 1:2],
                                     AF.Sqrt, bias=eps_sb)
                rstd2 = n1p.tile([128, 1], dt.float32, tag="rstd")
                nc.vector.reciprocal(out=rstd2[:], in_=stds[:, NT + w:NT + w + 1])
                z2 = n2p.tile([128, D], dt.bfloat16, tag="z2")
                nc.vector.tensor_scalar(out=z2[:], in0=x1[:],
                                        scalar1=mv2[:, 0:1], scalar2=rstd2[:],
                                        op0=OP.subtract, op1=OP.mult)
                tps2 = n2ps1.tile([128, 2, 128], dt.bfloat16, tag="tr")
                nc.tensor.transpose(tps2[:, 0, :], z2[:, 0:128], ident_sb)
                nc.tensor.transpose(tps2[:, 1, :], z2[:, 128:256], ident_sb)
                z2T = n2p.tile([128, 2, 128], dt.bfloat16, tag="z2T")
                nc.any.tensor_copy(out=z2T[:, 0, :], in_=tps2[:, 0, :])
                nc.any.tensor_copy(out=z2T[:, 1, :], in_=tps2[:, 1, :])
                for h in range(2):
                    f1ps = n2ps2.tile([128, 4, 128], dt.float32, tag="f1ps")
                    for s4 in range(4):
                        s = h * 4 + s4
                        nc.tensor.matmul(f1ps[:, s4, :],
                                         lhsT=wff1_k(0)[:, s * 128:(s + 1) * 128],
                                         rhs=z2T[:, 0, :], start=True, stop=False)
                        nc.tensor.matmul(f1ps[:, s4, :],
                                         lhsT=wff1_k(1)[:, s * 128:(s + 1) * 128],
                                         rhs=z2T[:, 1, :], start=False, stop=True)
                    for s4 in range(4):
                        s = h * 4 + s4
                        # silu(f1 + b_ff1) straight from PSUM; bias is
                        # per-partition in this [ff, node] layout
                        nc.scalar.activation(f1sil[:, s, :], f1ps[:, s4, :],
                                             AF.Silu, bias=bff1_sb[:, s:s + 1])
                f2ps = n2ps1.tile([128, D], dt.float32, tag="mm256")
                for s in range(8):
                    nc.tensor.matmul(f2ps[:], lhsT=f1sil[:, s, :],
                                     rhs=wff2_s(s),
                                     start=(s == 0), stop=(s == 7))
                if has_bff2:
                    o1 = n2p.tile([128, D], dt.float32, tag="o1")
                    nc.vector.tensor_tensor(out=o1[:], in0=f2ps[:], in1=bff2_sb,
                                            op=OP.add)
                    nc.vector.tensor_tensor(out=outb[:], in0=o1[:],
                                            in1=x1[:], op=OP.add)
                else:
                    nc.vector.tensor_tensor(out=outb[:], in0=f2ps[:],
                                            in1=x1[:], op=OP.add)
                nc.sync.dma_start(out=out_r[w], in_=outb[:])

    nc.compile()
    return nc


def _prep_inputs(x, radial_basis, src, dst, ln1_s, ln1_b, W_pre, b_pre,
                 W_filt, b_filt, W_post, b_post, ln2_s, ln2_b,
                 W_ff1, b_ff1, W_ff2, b_ff2):
    """Host-side staging: LN folds, edge sort/pad, per-core arrays."""
    x = _f32(x)
    rb = _f32(radial_basis)
    src = np.asarray(src).astype(np.int64)
    dst = np.asarray(dst).astype(np.int64)

    # fold LN1/LN2 scale+bias into the following Linear
    W_pre_f = _f32(ln1_s)[:, None] * _f32(W_pre)
    b_pre_f = _f32(ln1_b) @ _f32(W_pre) + _f32(b_pre)
    W_ff1_f = _f32(ln2_s)[:, None] * _f32(W_ff1)
    b_ff1_f = _f32(ln2_b) @ _f32(W_ff1) + _f32(b_ff1)

    order = np.argsort(dst, kind="stable")
    dst_s = dst[order]
    src_s = src[order]

    starts, tops = [], []
    for c in range(NCORES):
        for w in range(NWIN):
            starts.append(c * NPC + w * 128)
            tops.append(min(c * NPC + (w + 1) * 128, (c + 1) * NPC))
    edge_lo = np.searchsorted(dst_s, np.array(starts), side="left")
    edge_hi = np.searchsorted(dst_s, np.array(tops), side="left")

    # Pair edges whose srcs are consecutive node ids (s, s+1): one gather
    # descriptor fetches both h_pre rows. Regions per window: even-start
    # pairs, odd-start pairs, singles.
    percw = []
    for c in range(NCORES):
        rows = []
        for w in range(NWIN):
            k = c * NWIN + w
            lo, hi = edge_lo[k], edge_hi[k]
            e_src = src_s[lo:hi]
            e_dl = (dst_s[lo:hi] - (c * NPC + w * 128)).astype(np.int64)
            e_id = order[lo:hi]
            o2 = np.argsort(e_src, kind="stable")
            s2, d2, i2 = e_src[o2], e_dl[o2], e_id[o2]
            u, first_idx = np.unique(s2, return_index=True)
            pe, po = [], []
            i = 0
            while i < len(u) - 1:
                if u[i + 1] == u[i] + 1:
                    (pe if u[i] % 2 == 0 else po).append(
                        (int(u[i]), int(first_idx[i]), int(first_idx[i + 1])))
                    i += 2
                else:
                    i += 1
            used = np.zeros(len(s2), bool)
            for (_, plo, phi) in pe + po:
                used[plo] = used[phi] = True
            si = np.nonzero(~used)[0]
            rows.append(dict(pe=pe, po=po, si=si, s2=s2, d2=d2, i2=i2))
        percw.append(rows)

    def _cdiv(a, b):
        return -(-a // b)

    BPe = max(1, max(_cdiv(len(percw[c][w]["pe"]), 128)
                     for c in range(NCORES) for w in range(NWIN)))
    BPo = max(1, max(_cdiv(len(percw[c][w]["po"]), 128)
                     for c in range(NCORES) for w in range(NWIN)))
    BS = max(1, max(_cdiv(len(percw[c][w]["si"]), 128)
                    for c in range(NCORES) for w in range(NWIN)))
    mp = []
    for w in range(NWIN):
        mpe = min(len(percw[c][w]["pe"]) // 128 for c in range(NCORES))
        mpo = min(len(percw[c][w]["po"]) // 128 for c in range(NCORES))
        msi = min(len(percw[c][w]["si"]) // 128 for c in range(NCORES))
        mp.append([min(mpe, BPe), min(mpo, BPo), min(msi, BS)])
    geo = dict(BPe=BPe, BPo=BPo, BS=BS, mp=mp)
    Bw = 2 * BPe + 2 * BPo + BS
    NIW = (BPe + BPo + BS) * 128
    EPW = Bw * 128
    NBLK = NWIN * Bw

    flags = dict(
        bfilt=bool(np.any(np.asarray(b_filt) != 0)),
        bpre=bool(np.any(b_pre_f != 0)),
        bpost=bool(np.any(np.asarray(b_post) != 0)),
        bff2=bool(np.any(np.asarray(b_ff2) != 0)),
    )

    wpre_bf = _bf(W_pre_f)
    cpack_common = np.concatenate([
        np.broadcast_to(b_pre_f, (128, DH)),
        np.broadcast_to(_f32(b_post), (128, D)),
        np.broadcast_to(_f32(b_ff2), (128, D)),
        np.broadcast_to(np.arange(128, dtype=np.float32), (128, 128)),
        np.ascontiguousarray(b_ff1_f.reshape(8, 128).T),
        np.full((128, 1), EPS, dtype=np.float32),
    ], axis=1).astype(np.float32)

    wpack = np.concatenate([
        wpre_bf[0:128], wpre_bf[128:256],
        _bf(W_filt),
        _bf(W_post)[0:128], _bf(W_post)[128:256],
        _bf(W_ff1_f)[0:128], _bf(W_ff1_f)[128:256],
        np.concatenate([_bf(W_ff2)[s * 128:(s + 1) * 128] for s in range(8)],
                       axis=1).reshape(128, 8 * D),
        _bf(np.eye(128, dtype=np.float32)),
    ], axis=1).astype(BF16)
    assert wpack.shape == (128, W_TOT), wpack.shape

    per_core = []
    for c in range(NCORES):
        idx_pad = np.zeros((NWIN, NIW), dtype=np.int64)
        dl_pad = np.full((NWIN, EPW), -1, dtype=np.int64)
        eids = np.full((NWIN, EPW), -1, dtype=np.int64)
        for w in range(NWIN):
            r = percw[c][w]
            s2, d2, i2 = r["s2"], r["d2"], r["i2"]
            # paired regions
            for ri, (plist, base_blk, idx_base) in enumerate(
                    [(r["pe"], 0, 0), (r["po"], 2 * BPe, BPe * 128)]):
                for pr, (s, plo, phi) in enumerate(plist):
                    blk = base_blk + 2 * (pr // 128)
                    p = pr % 128
                    slot_lo = blk * 128 + p
                    slot_hi = (blk + 1) * 128 + p
                    eids[w, slot_lo] = i2[plo]
                    eids[w, slot_hi] = i2[phi]
                    dl_pad[w, slot_lo] = d2[plo]
                    dl_pad[w, slot_hi] = d2[phi]
                    idx_pad[w, idx_base + pr] = s // 2 if ri == 0 else (s - 1) // 2
            # singles region
            base2 = (2 * BPe + 2 * BPo) * 128
            si = r["si"]
            eids[w, base2:base2 + len(si)] = i2[si]
            dl_pad[w, base2:base2 + len(si)] = d2[si]
            idx_pad[w, (BPe + BPo) * 128:(BPe + BPo) * 128 + len(si)] = s2[si]

        flat_eids = eids.reshape(-1)
        rb_rows = np.zeros((NWIN * EPW, DR), dtype=np.float32)
        valid = flat_eids >= 0
        rb_rows[valid] = rb[flat_eids[valid]]
        rbT = np.ascontiguousarray(rb_rows.T).astype(BF16)

        gi = np.zeros((NWIN, 128, NIW // 16), dtype=np.int16)
        for w in range(NWIN):
            wrapped = idx_pad[w].reshape(NIW // 16, 16).T.astype(np.int16)
            gi[w] = np.tile(wrapped, (8, 1))

        # one-hot scatter matrices, streamed as [128, NBLK*128] bf16:
        # block j cols [j*128,(j+1)*128); row p = edge p of block j
        dflat = dl_pad.reshape(NBLK * 128)  # [j*128+p]
        ohp = np.zeros((128, NBLK * 128), dtype=BF16)
        eidx = np.nonzero(dflat >= 0)[0]
        jj = eidx // 128
        pp = eidx % 128
        ohp[pp, jj * 128 + dflat[eidx]] = 1.0

        xr = np.zeros((NWIN * 128, D), dtype=np.float32)
        xr[:NPC] = x[c * NPC:(c + 1) * NPC]

        parts = [cpack_common]
        if flags["bfilt"]:
            parts.append(np.broadcast_to(_f32(b_filt), (128, DH)))
        cpack = _f32(np.concatenate(parts, axis=1))

        per_core.append(dict(rbT=rbT, gidx=gi, cpack=cpack, xres=xr, ohp=ohp))

    xpad = np.zeros((NPAD, D), dtype=np.float32)
    xpad[:N_NODES] = x
    consts = dict(xnm=_bf(xpad), wpack=wpack)
    return geo, flags, consts, per_core


LAST_EXEC_TIME_NS = None
LAST_RESULTS = None


def kernel(**inputs) -> np.ndarray:
    global LAST_EXEC_TIME_NS, LAST_RESULTS
    geo, flags, consts, per_core = _prep_inputs(**inputs)
    nc = _build_program(geo, flags)
    in_maps = []
    for c in range(NCORES):
        m = dict(consts)
        m.update(per_core[c])
        in_maps.append(m)
    res = bass_utils.run_bass_kernel_spmd(nc, in_maps, list(range(NCORES)))
    LAST_EXEC_TIME_NS = getattr(res, "exec_time_ns", None)
    LAST_RESULTS = res
    out = np.concatenate(
        [res.results[c]["out"][:NPC] for c in range(NCORES)], axis=0
    )
    return np.ascontiguousarray(out, dtype=np.float32)


# revision 46
# speedup vs baseline: 1.1684x; 1.1684x over previous
"""CFBlock (GNN message passing) Trainium2 Bass kernel.

Sharding: edges sorted by dst; each of the 8 cores owns a contiguous range of
1250 destination nodes and all edges pointing into it. Each core:
  - (replicated) computes h_pre = LN1(x) @ W_pre + b_pre for ALL nodes and
    stores it as a bf16 table in DRAM,
  - gathers h_pre[src] for its edges with dma_gather, computes the edge filter
    GEMM, multiplies, and segment-sums via one-hot matmuls into PSUM windows
    of 128 destination nodes,
  - runs post-Linear + SiLU + residual + LN2 + FFN + residual for its nodes.
No collectives; the host concatenates the 8 output slices.

V2 notes (post-trace):
  - bulk DMAs (rbT, h_pre, out) ride HWDGE (nc.sync) so GpSimd is gather-only
  - one-hot built with 2-op tensor_scalar (subtract, is_equal) -- the 1-op
    is_equal encoding measured ~1us on HW
  - filter GEMM runs 2 blocks per PSUM bank; PSUM->SBUF casts ride nc.any
    (scheduler balances ACT/DVE); edge multiply is bf16 TT at 2x
  - gathers skip trailing pad rows (idx -1) with memset-zeroed tails
  - zero biases (the common case) skip their add instructions entirely;
    SiLU runs straight from PSUM on the scalar engine with fused bias
"""

import sys
import types

import numpy as np
import ml_dtypes

import concourse.bass as bass
import concourse.mybir as mybir
from concourse import bacc
from concourse import library_config
from concourse.tile import TileContext
from concourse import bass_utils

# Defensive: if BASS_TRACE is set but the image's antenv lacks axon_hooks,
# run_bass_kernel_spmd would crash on import. Provide a null hook module so
# bass_utils falls back to running untraced.
if "antenv.axon_hooks" not in sys.modules:
    try:
        import antenv

        _m = types.ModuleType("antenv.axon_hooks")
        _m._hook = None
        _m.set_axon_ntff_profile_hook = lambda h: setattr(_m, "_hook", h)
        _m.get_axon_ntff_profile_hook = lambda: _m._hook
        sys.modules["antenv.axon_hooks"] = _m
        antenv.axon_hooks = _m
    except ImportError:
        pass

BF16 = ml_dtypes.bfloat16

N_NODES = 10000
N_EDGES = 320000
D = 256          # d_model
DR = 128         # d_radial
DH = 256         # d_hidden
DFF = 1024
EPS = 1e-5
NCORES = 8
NPC = 1250       # nodes per core
NWIN = 10        # 128-node windows per core (last window: 98 valid nodes)
NPAD = 10112     # 79 * 128
NT = NPAD // 128  # 79 node tiles
XCH = 16         # node tiles per xT DMA chunk
RBCH = 16        # edge blocks per rbT DMA chunk
HB = 16          # h_pre tiles batched per DMA write

AF = mybir.ActivationFunctionType
OP = mybir.AluOpType

# packed f32 const columns: bpre, bpost, bff2, iota, bff1, eps, dstloc[, bfilt]
C_BPRE, C_BPOST, C_BFF2 = 0, 256, 512
C_IOTA, C_BFF1, C_EPS, C_DSTLOC = 768, 896, 904, 905
# packed bf16 const columns
W_PRE, W_FILT, W_POST, W_FF1, W_FF2, W_ID = 0, 512, 768, 1280, 3328, 5376
W_TOT = 5504


def _f32(a):
    return np.ascontiguousarray(a, dtype=np.float32)


def _bf(a):
    return np.ascontiguousarray(np.asarray(a, dtype=np.float32).astype(BF16))


def _build_program(geo: dict, flags: dict, phase: int = 4):
    nc = bacc.Bacc("TRN2", target_bir_lowering=False, debug=False)
    dt = mybir.dt

    has_bfilt = flags["bfilt"]
    has_bpre = flags["bpre"]
    has_bpost = flags["bpost"]
    has_bff2 = flags["bff2"]

    BPe, BPo, BS = geo["BPe"], geo["BPo"], geo["BS"]
    mp = geo["mp"]            # [NWIN][3] first block to memset per region
    Bw = 2 * BPe + 2 * BPo + BS   # edge blocks per window
    NIW = (BPe + BPo + BS) * 128  # gather indices per window
    EPW = Bw * 128            # padded edges per window
    NBLK = NWIN * Bw          # padded blocks per core
    CW = C_EPS + 1 + (DH if has_bfilt else 0)
    C_BFILT = C_EPS + 1

    # ---- I/O ----
    xnm_d = nc.dram_tensor("xnm", [NPAD, D], dt.bfloat16, kind="ExternalInput")
    xres_d = nc.dram_tensor("xres", [NWIN * 128, D], dt.float32, kind="ExternalInput")
    rbT_d = nc.dram_tensor("rbT", [DR, NBLK * 128], dt.bfloat16, kind="ExternalInput")
    ohp_d = nc.dram_tensor("ohp", [128, NBLK * 128], dt.bfloat16, kind="ExternalInput")
    gidx_d = nc.dram_tensor("gidx", [NWIN, 128, NIW // 16], dt.int16, kind="ExternalInput")
    cpack_d = nc.dram_tensor("cpack", [128, CW], dt.float32, kind="ExternalInput")
    wpack_d = nc.dram_tensor("wpack", [128, W_TOT], dt.bfloat16, kind="ExternalInput")
    out_d = nc.dram_tensor("out", [NWIN * 128, D], dt.float32, kind="ExternalOutput")

    with TileContext(nc) as tc:
        with (
            tc.tile_pool(name="consts", bufs=1) as consts,
            tc.tile_pool(name="dram", bufs=1, space="DRAM") as dramp,
            tc.tile_pool(name="n1", bufs=12) as n1p,
            tc.tile_pool(name="hout", bufs=2) as houtp,
            tc.tile_pool(name="rbt", bufs=3) as rbtp,
            tc.tile_pool(name="ohp", bufs=3) as ohpp,
            tc.tile_pool(name="gp", bufs=3) as gpp,
            tc.tile_pool(name="gs", bufs=2) as gsp,
            tc.tile_pool(name="edge", bufs=10) as edgep,
            # PSUM budget is 8 banks of 2KB/partition, sized to exactly 8:
            # fps(2) + hagg/pps(2) + tr(1) + mm256(1) + f1ps(2|1) + gagg(0|1)
            tc.tile_pool(name="fps", bufs=2, space="PSUM") as fpsp,
            tc.tile_pool(name="hagg", bufs=2, space="PSUM") as haggp,
            tc.tile_pool(name="n2", bufs=2) as n2p,
            tc.tile_pool(name="n2ps1", bufs=1, space="PSUM") as n2ps1,
            tc.tile_pool(name="n2ps2", bufs=1 if has_bfilt else 2,
                         space="PSUM") as n2ps2,
            tc.tile_pool(name="gaggp", bufs=1, space="PSUM") as gaggp,
        ):
            nc.gpsimd.load_library(library_config.mlp)
            cpk = consts.tile([128, CW], dt.float32, tag="cpack")
            nc.sync.dma_start(out=cpk[:], in_=cpack_d[:])
            wpk = consts.tile([128, W_TOT], dt.bfloat16, tag="wpack")
            nc.sync.dma_start(out=wpk[:], in_=wpack_d[:])
            gidx_sb = consts.tile([128, NWIN, NIW // 16], dt.int16, tag="gidx")
            nc.sync.dma_start(out=gidx_sb[:],
                              in_=gidx_d[:].rearrange("w p s -> p w s"))
            xnm_r = xnm_d[:].rearrange("(t p) n -> t p n", p=128)
            xnm_sb = consts.tile([128, NT, D], dt.bfloat16, tag="xbig")
            nc.sync.dma_start(out=xnm_sb[:],
                              in_=xnm_r.rearrange("t p n -> p t n"))
            xres_r = xres_d[:].rearrange("(w p) n -> w p n", p=128)
            xres_sb = consts.tile([128, NWIN, D], dt.float32, tag="xresb")
            nc.sync.dma_start(out=xres_sb[:],
                              in_=xres_r.rearrange("w p n -> p w n"))
            stds = consts.tile([128, NT + NWIN], dt.float32, tag="stds")

            bpre_sb = cpk[:, C_BPRE:C_BPRE + DH]
            bpost_sb = cpk[:, C_BPOST:C_BPOST + D]
            bff2_sb = cpk[:, C_BFF2:C_BFF2 + D]

            bff1_sb = cpk[:, C_BFF1:C_BFF1 + 8]
            eps_sb = cpk[:, C_EPS:C_EPS + 1]

            bfilt_sb = cpk[:, C_BFILT:C_BFILT + DH] if has_bfilt else None
            wpre_k = lambda k: wpk[:, W_PRE + k * DH:W_PRE + (k + 1) * DH]
            wfilt_sb = wpk[:, W_FILT:W_FILT + DH]
            wpost_k = lambda k: wpk[:, W_POST + k * D:W_POST + (k + 1) * D]
            wff1_k = lambda k: wpk[:, W_FF1 + k * DFF:W_FF1 + (k + 1) * DFF]
            wff2_s = lambda s: wpk[:, W_FF2 + s * D:W_FF2 + (s + 1) * D]
            ident_sb = wpk[:, W_ID:W_ID + 128]

            hpre_dram = dramp.tile([NPAD, D], dt.bfloat16, tag="hpre")
            hpre_r = hpre_dram[:].rearrange("(t p) n -> t p n", p=128)

            # ---- node phase 1: h_pre for all nodes ----
            h_big = None
            for t in range(NT):
                if t % HB == 0:
                    h_big = houtp.tile([128, HB, DH], dt.bfloat16, tag="hsb")
                x_sb = xnm_sb[:, t, :]
                stats = n1p.tile([128, 6], dt.float32, tag="bnst")
                nc.vector.bn_stats(out=stats[:], in_=x_sb)
                mv = n1p.tile([128, 2], dt.float32, tag="bnagg")
                nc.vector.bn_aggr(out=mv[:], in_=stats[:])
                nc.scalar.activation(stds[:, t:t + 1], mv[:, 1:2], AF.Sqrt,
                                     bias=eps_sb)
                rstd = n1p.tile([128, 1], dt.float32, tag="rstd")
                nc.vector.reciprocal(out=rstd[:], in_=stds[:, t:t + 1])
                z = n1p.tile([128, D], dt.bfloat16, tag="z")
                nc.vector.tensor_scalar(out=z[:], in0=x_sb,
                                        scalar1=mv[:, 0:1], scalar2=rstd[:],
                                        op0=OP.subtract, op1=OP.mult)
                ztps = n2ps1.tile([128, 2, 128], dt.bfloat16, tag="tr")
                nc.tensor.transpose(ztps[:, 0, :], z[:, 0:128], ident_sb)
                nc.tensor.transpose(ztps[:, 1, :], z[:, 128:256], ident_sb)
                zT = n1p.tile([128, 2, 128], dt.bfloat16, tag="zT")
                nc.any.tensor_copy(out=zT[:, 0, :], in_=ztps[:, 0, :])
                nc.any.tensor_copy(out=zT[:, 1, :], in_=ztps[:, 1, :])
                pps = haggp.tile([128, DH], dt.float32, tag="hagg")
                nc.tensor.matmul(pps[:], lhsT=zT[:, 0, :],
                                 rhs=wpre_k(0), start=True, stop=False)
                nc.tensor.matmul(pps[:], lhsT=zT[:, 1, :],
                                 rhs=wpre_k(1), start=False, stop=True)
                if has_bpre:
                    nc.vector.tensor_tensor(out=h_big[:, t % HB, :], in0=pps[:],
                                            in1=bpre_sb, op=OP.add)
                else:
                    nc.any.tensor_copy(out=h_big[:, t % HB, :], in_=pps[:])
                if t % HB == HB - 1 or t == NT - 1:
                    t0 = (t // HB) * HB
                    nb = t - t0 + 1
                    nc.sync.dma_start(
                        out=hpre_r[t0:t0 + nb].rearrange("t p n -> p t n"),
                        in_=h_big[:, :nb, :])

            # ---- edge phase + per-window epilogue ----
            out_r = out_d[:].rearrange("(w p) n -> w p n", p=128)
            # gather source views: pair rows fetch 2 consecutive h_pre rows
            hpe_v = hpre_dram[0:NPAD, :].rearrange("(a two) n -> a (two n)",
                                                   two=2)
            hpo_v = hpre_dram[1:NPAD - 1, :].rearrange(
                "(a two) n -> a (two n)", two=2)
            regions = [(BPe, 512, hpe_v), (BPo, 512, hpo_v),
                       (BS, 256, hpre_dram[:])]
            rbt_chunks = {}
            oh_chunks = {}

            def get_rbt(j):
                k = j // RBCH
                if k not in rbt_chunks:
                    c0 = k * RBCH * 128
                    ncols = min(RBCH * 128, NBLK * 128 - c0)
                    t = rbtp.tile([128, RBCH * 128], dt.bfloat16, tag="rbt")
                    nc.sync.dma_start(out=t[:, :ncols],
                                      in_=rbT_d[:, c0:c0 + ncols])
                    rbt_chunks[k] = t
                return rbt_chunks[k]

            def get_oh(j):
                k = j // RBCH
                if k not in oh_chunks:
                    c0 = k * RBCH * 128
                    ncols = min(RBCH * 128, NBLK * 128 - c0)
                    t = ohpp.tile([128, RBCH * 128], dt.bfloat16, tag="ohp")
                    nc.sync.dma_start(out=t[:, :ncols],
                                      in_=ohp_d[:, c0:c0 + ncols])
                    oh_chunks[k] = t
                return oh_chunks[k]
            for w in range(NWIN if phase >= 2 else 0):
                g_views = []
                offc = 0  # column offset into gidx_sb (16-wrapped idx units)
                for ri, (RB, esz, src_view) in enumerate(regions):
                    pool = gsp if ri == 2 else gpp
                    gt = pool.tile([128, RB, esz], dt.bfloat16, tag=f"g{ri}")
                    nc.gpsimd.dma_gather(
                        gt[:], src_view,
                        gidx_sb[:, w, offc:offc + RB * 8],
                        RB * 128, RB * 128, esz, single_packet=False)
                    offc += RB * 8
                    nb_view = RB * esz // DH
                    g_views.append(
                        (gt[:].rearrange("p b (k n) -> p (b k) n", n=DH),
                         nb_view))
                if phase < 3:
                    continue
                hagg = haggp.tile([128, DH], dt.float32, tag="hagg")
                gagg = None
                if has_bfilt:
                    gagg = gaggp.tile([128, DH], dt.float32, tag="gagg")
                jw = w * Bw  # running global block index
                for g_view, nbv in g_views:
                    for p in range((nbv + 1) // 2):
                        b0g = 2 * p
                        nb = min(2, nbv - b0g)
                        j0 = jw + b0g
                        # filter GEMM for up to two blocks into one PSUM bank
                        fps = fpsp.tile([128, 2, DH], dt.float32, tag="fps")
                        for i in range(nb):
                            rbt_sb = get_rbt(j0 + i)
                            boff = ((j0 + i) % RBCH) * 128
                            nc.tensor.matmul(fps[:, i, :],
                                             lhsT=rbt_sb[:, boff:boff + 128],
                                             rhs=wfilt_sb, start=True, stop=True)
                        fsb = edgep.tile([128, 2, DH], dt.bfloat16, tag="fsb")
                        nc.any.tensor_copy(out=fsb[:, :nb, :], in_=fps[:, :nb, :])
                        m_sb = edgep.tile([128, 2, DH], dt.bfloat16, tag="m")
                        nc.vector.tensor_tensor(
                            out=m_sb[:, :nb, :], in0=fsb[:, :nb, :],
                            in1=g_view[:, b0g:b0g + nb, :], op=OP.mult)
                        for i in range(nb):
                            j = j0 + i
                            oh_sb = get_oh(j)
                            boff = (j % RBCH) * 128
                            oh = oh_sb[:, boff:boff + 128]
                            nc.tensor.matmul(hagg[:], lhsT=oh,
                                             rhs=m_sb[:, i, :],
                                             start=(j == w * Bw),
                                             stop=(j == w * Bw + Bw - 1))
                            if has_bfilt:
                                nc.tensor.matmul(gagg[:], lhsT=oh,
                                                 rhs=g_view[:, b0g + i, :],
                                                 start=(j == w * Bw),
                                                 stop=(j == w * Bw + Bw - 1))
                    jw += nbv

                if phase < 4:
                    continue
                # ---- epilogue for this window ----
                outb = n2p.tile([128, D], dt.float32, tag="outb")
                f1sil = n2p.tile([128, 8, 128], dt.bfloat16, tag="f1sil")
                hagg_sb = n2p.tile([128, DH], dt.bfloat16, tag="haggsb")
                if has_bfilt:
                    tmpb = n2p.tile([128, DH], dt.float32, tag="tmpb")
                    nc.vector.tensor_tensor(out=tmpb[:], in0=gagg[:],
                                            in1=bfilt_sb, op=OP.mult)
                    nc.vector.tensor_tensor(out=hagg_sb[:], in0=hagg[:],
                                            in1=tmpb[:], op=OP.add)
                else:
                    nc.any.tensor_copy(out=hagg_sb[:], in_=hagg[:])
                tps = n2ps1.tile([128, 2, 128], dt.bfloat16, tag="tr")
                nc.tensor.transpose(tps[:, 0, :], hagg_sb[:, 0:128], ident_sb)
                nc.tensor.transpose(tps[:, 1, :], hagg_sb[:, 128:256], ident_sb)
                haggT = n2p.tile([128, 2, 128], dt.bfloat16, tag="haggT")
                nc.any.tensor_copy(out=haggT[:, 0, :], in_=tps[:, 0, :])
                nc.any.tensor_copy(out=haggT[:, 1, :], in_=tps[:, 1, :])
                pops = n2ps1.tile([128, D], dt.float32, tag="mm256")
                nc.tensor.matmul(pops[:], lhsT=haggT[:, 0, :],
                                 rhs=wpost_k(0), start=True, stop=False)
                nc.tensor.matmul(pops[:], lhsT=haggT[:, 1, :],
                                 rhs=wpost_k(1), start=False, stop=True)
                if has_bpost:
                    ps_sb = n2p.tile([128, D], dt.float32, tag="pssb")
                    nc.vector.tensor_tensor(out=ps_sb[:], in0=pops[:],
                                            in1=bpost_sb, op=OP.add)
                    nc.scalar.activation(outb[:], ps_sb[:], AF.Silu)
                else:
                    nc.scalar.activation(outb[:], pops[:], AF.Silu)
                x1 = n2p.tile([128, D], dt.float32, tag="x1")
                nc.vector.tensor_tensor(out=x1[:], in0=outb[:],
                                        in1=xres_sb[:, w, :], op=OP.add)
                # LN2
                st2 = n1p.tile([128, 6], dt.float32, tag="bnst")
                nc.vector.bn_stats(out=st2[:], in_=x1[:])
                mv2 = n1p.tile([128, 2], dt.float32, tag="bnagg")
                nc.vector.bn_aggr(out=mv2[:], in_=st2[:])
                nc.scalar.activation(stds[:, NT + w:NT + w + 1], mv2[:, 1:2],
                                     AF.Sqrt, bias=eps_sb)
                rstd2 = n1p.tile([128, 1], dt.float32, tag="rstd")
                nc.vector.reciprocal(out=rstd2[:], in_=stds[:, NT + w:NT + w + 1])
                z2 = n2p.tile([128, D], dt.bfloat16, tag="z2")
                nc.vector.tensor_scalar(out=z2[:], in0=x1[:],
                                        scalar1=mv2[:, 0:1], scalar2=rstd2[:],
                                        op0=OP.subtract, op1=OP.mult)
                tps2 = n2ps1.tile([128, 2, 128], dt.bfloat16, tag="tr")
                nc.tensor.transpose(tps2[:, 0, :], z2[:, 0:128], ident_sb)
                nc.tensor.transpose(tps2[:, 1, :], z2[:, 128:256], ident_sb)
                z2T = n2p.tile([128, 2, 128], dt.bfloat16, tag="z2T")
                nc.any.tensor_copy(out=z2T[:, 0, :], in_=tps2[:, 0, :])
                nc.any.tensor_copy(out=z2T[:, 1, :], in_=tps2[:, 1, :])
                for h in range(2):
                    f1ps = n2ps2.tile([128, 4, 128], dt.float32, tag="f1ps")
                    for s4 in range(4):
                        s = h * 4 + s4
                        nc.tensor.matmul(f1ps[:, s4, :],
                                         lhsT=wff1_k(0)[:, s * 128:(s + 1) * 128],
                                         rhs=z2T[:, 0, :], start=True, stop=False)
                        nc.tensor.matmul(f1ps[:, s4, :],
                                         lhsT=wff1_k(1)[:, s * 128:(s + 1) * 128],
                                         rhs=z2T[:, 1, :], start=False, stop=True)
                    for s4 in range(4):
                        s = h * 4 + s4
                        # silu(f1 + b_ff1) straight from PSUM; bias is
                        # per-partition in this [ff, node] layout
                        nc.scalar.activation(f1sil[:, s, :], f1ps[:, s4, :],
                                             AF.Silu, bias=bff1_sb[:, s:s + 1])
                f2ps = n2ps1.tile([128, D], dt.float32, tag="mm256")
                for s in range(8):
                    nc.tensor.matmul(f2ps[:], lhsT=f1sil[:, s, :],
                                     rhs=wff2_s(s),
                                     start=(s == 0), stop=(s == 7))
                if has_bff2:
                    o1 = n2p.tile([128, D], dt.float32, tag="o1")
                    nc.vector.tensor_tensor(out=o1[:], in0=f2ps[:], in1=bff2_sb,
                                            op=OP.add)
                    nc.vector.tensor_tensor(out=outb[:], in0=o1[:],
                                            in1=x1[:], op=OP.add)
                else:
                    nc.vector.tensor_tensor(out=outb[:], in0=f2ps[:],
                                            in1=x1[:], op=OP.add)
                nc.sync.dma_start(out=out_r[w], in_=outb[:])

    nc.compile()
    return nc


def _prep_inputs(x, radial_basis, src, dst, ln1_s, ln1_b, W_pre, b_pre,
                 W_filt, b_filt, W_post, b_post, ln2_s, ln2_b,
                 W_ff1, b_ff1, W_ff2, b_ff2):
    """Host-side staging: LN folds, edge sort/pad, per-core arrays."""
    x = _f32(x)
    rb = _f32(radial_basis)
    src = np.asarray(src).astype(np.int64)
    dst = np.asarray(dst).astype(np.int64)

    # fold LN1/LN2 scale+bias into the following Linear
    W_pre_f = _f32(ln1_s)[:, None] * _f32(W_pre)
    b_pre_f = _f32(ln1_b) @ _f32(W_pre) + _f32(b_pre)
    W_ff1_f = _f32(ln2_s)[:, None] * _f32(W_ff1)
    b_ff1_f = _f32(ln2_b) @ _f32(W_ff1) + _f32(b_ff1)

    order = np.argsort(dst, kind="stable")
    dst_s = dst[order]
    src_s = src[order]

    starts, tops = [], []
    for c in range(NCORES):
        for w in range(NWIN):
            starts.append(c * NPC + w * 128)
            tops.append(min(c * NPC + (w + 1) * 128, (c + 1) * NPC))
    edge_lo = np.searchsorted(dst_s, np.array(starts), side="left")
    edge_hi = np.searchsorted(dst_s, np.array(tops), side="left")

    # Pair edges whose srcs are consecutive node ids (s, s+1): one gather
    # descriptor fetches both h_pre rows. Regions per window: even-start
    # pairs, odd-start pairs, singles.
    percw = []
    for c in range(NCORES):
        rows = []
        for w in range(NWIN):
            k = c * NWIN + w
            lo, hi = edge_lo[k], edge_hi[k]
            e_src = src_s[lo:hi]
            e_dl = (dst_s[lo:hi] - (c * NPC + w * 128)).astype(np.int64)
            e_id = order[lo:hi]
            o2 = np.argsort(e_src, kind="stable")
            s2, d2, i2 = e_src[o2], e_dl[o2], e_id[o2]
            u, first_idx = np.unique(s2, return_index=True)
            pe, po = [], []
            i = 0
            while i < len(u) - 1:
                if u[i + 1] == u[i] + 1:
                    (pe if u[i] % 2 == 0 else po).append(
                        (int(u[i]), int(first_idx[i]), int(first_idx[i + 1])))
                    i += 2
                else:
                    i += 1
            used = np.zeros(len(s2), bool)
            for (_, plo, phi) in pe + po:
                used[plo] = used[phi] = True
            si = np.nonzero(~used)[0]
            rows.append(dict(pe=pe, po=po, si=si, s2=s2, d2=d2, i2=i2))
        percw.append(rows)

    def _cdiv(a, b):
        return -(-a // b)

    BPe = max(1, max(_cdiv(len(percw[c][w]["pe"]), 128)
                     for c in range(NCORES) for w in range(NWIN)))
    BPo = max(1, max(_cdiv(len(percw[c][w]["po"]), 128)
                     for c in range(NCORES) for w in range(NWIN)))
    BS = max(1, max(_cdiv(len(percw[c][w]["si"]), 128)
                    for c in range(NCORES) for w in range(NWIN)))
    mp = []
    for w in range(NWIN):
        mpe = min(len(percw[c][w]["pe"]) // 128 for c in range(NCORES))
        mpo = min(len(percw[c][w]["po"]) // 128 for c in range(NCORES))
        msi = min(len(percw[c][w]["si"]) // 128 for c in range(NCORES))
        mp.append([min(mpe, BPe), min(mpo, BPo), min(msi, BS)])
    geo = dict(BPe=BPe, BPo=BPo, BS=BS, mp=mp)
    Bw = 2 * BPe + 2 * BPo + BS
    NIW = (BPe + BPo + BS) * 128
    EPW = Bw * 128
    NBLK = NWIN * Bw

    flags = dict(
        bfilt=bool(np.any(np.asarray(b_filt) != 0)),
        bpre=bool(np.any(b_pre_f != 0)),
        bpost=bool(np.any(np.asarray(b_post) != 0)),
        bff2=bool(np.any(np.asarray(b_ff2) != 0)),
    )

    wpre_bf = _bf(W_pre_f)
    cpack_common = np.concatenate([
        np.broadcast_to(b_pre_f, (128, DH)),
        np.broadcast_to(_f32(b_post), (128, D)),
        np.broadcast_to(_f32(b_ff2), (128, D)),
        np.broadcast_to(np.arange(128, dtype=np.float32), (128, 128)),
        np.ascontiguousarray(b_ff1_f.reshape(8, 128).T),
        np.full((128, 1), EPS, dtype=np.float32),
    ], axis=1).astype(np.float32)

    wpack = np.concatenate([
        wpre_bf[0:128], wpre_bf[128:256],
        _bf(W_filt),
        _bf(W_post)[0:128], _bf(W_post)[128:256],
        _bf(W_ff1_f)[0:128], _bf(W_ff1_f)[128:256],
        np.concatenate([_bf(W_ff2)[s * 128:(s + 1) * 128] for s in range(8)],
                       axis=1).reshape(128, 8 * D),
        _bf(np.eye(128, dtype=np.float32)),
    ], axis=1).astype(BF16)
    assert wpack.shape == (128, W_TOT), wpack.shape

    per_core = []
    for c in range(NCORES):
        idx_pad = np.zeros((NWIN, NIW), dtype=np.int64)
        dl_pad = np.full((NWIN, EPW), -1, dtype=np.int64)
        eids = np.full((NWIN, EPW), -1, dtype=np.int64)
        for w in range(NWIN):
            r = percw[c][w]
            s2, d2, i2 = r["s2"], r["d2"], r["i2"]
            # paired regions
            for ri, (plist, base_blk, idx_base) in enumerate(
                    [(r["pe"], 0, 0), (r["po"], 2 * BPe, BPe * 128)]):
                for pr, (s, plo, phi) in enumerate(plist):
                    blk = base_blk + 2 * (pr // 128)
                    p = pr % 128
                    slot_lo = blk * 128 + p
                    slot_hi = (blk + 1) * 128 + p
                    eids[w, slot_lo] = i2[plo]
                    eids[w, slot_hi] = i2[phi]
                    dl_pad[w, slot_lo] = d2[plo]
                    dl_pad[w, slot_hi] = d2[phi]
                    idx_pad[w, idx_base + pr] = s // 2 if ri == 0 else (s - 1) // 2
            # singles region
            base2 = (2 * BPe + 2 * BPo) * 128
            si = r["si"]
            eids[w, base2:base2 + len(si)] = i2[si]
            dl_pad[w, base2:base2 + len(si)] = d2[si]
            idx_pad[w, (BPe + BPo) * 128:(BPe + BPo) * 128 + len(si)] = s2[si]

        flat_eids = eids.reshape(-1)
        rb_rows = np.zeros((NWIN * EPW, DR), dtype=np.float32)
        valid = flat_eids >= 0
        rb_rows[valid] = rb[flat_eids[valid]]
        rbT = np.ascontiguousarray(rb_rows.T).astype(BF16)

        gi = np.zeros((NWIN, 128, NIW // 16), dtype=np.int16)
        for w in range(NWIN):
            wrapped = idx_pad[w].reshape(NIW // 16, 16).T.astype(np.int16)
            gi[w] = np.tile(wrapped, (8, 1))

        # one-hot scatter matrices, streamed as [128, NBLK*128] bf16:
        # block j cols [j*128,(j+1)*128); row p = edge p of block j
        dflat = dl_pad.reshape(NBLK * 128)  # [j*128+p]
        ohp = np.zeros((128, NBLK * 128), dtype=BF16)
        eidx = np.nonzero(dflat >= 0)[0]
        jj = eidx // 128
        pp = eidx % 128
        ohp[pp, jj * 128 + dflat[eidx]] = 1.0

        xr = np.zeros((NWIN * 128, D), dtype=np.float32)
        xr[:NPC] = x[c * NPC:(c + 1) * NPC]

        parts = [cpack_common]
        if flags["bfilt"]:
            parts.append(np.broadcast_to(_f32(b_filt), (128, DH)))
        cpack = _f32(np.concatenate(parts, axis=1))

        per_core.append(dict(rbT=rbT, gidx=gi, cpack=cpack, xres=xr, ohp=ohp))

    xpad = np.zeros((NPAD, D), dtype=np.float32)
    xpad[:N_NODES] = x
    consts = dict(xnm=_bf(xpad), wpack=wpack)
    return geo, flags, consts, per_core


LAST_EXEC_TIME_NS = None
LAST_RESULTS = None


def kernel(**inputs) -> np.ndarray:
    global LAST_EXEC_TIME_NS, LAST_RESULTS
    geo, flags, consts, per_core = _prep_inputs(**inputs)
    nc = _build_program(geo, flags)
    in_maps = []
    for c in range(NCORES):
        m = dict(consts)
        m.update(per_core[c])
        in_maps.append(m)
    res = bass_utils.run_bass_kernel_spmd(nc, in_maps, list(range(NCORES)))
    LAST_EXEC_TIME_NS = getattr(res, "exec_time_ns", None)
    LAST_RESULTS = res
    out = np.concatenate(
        [res.results[c]["out"][:NPC] for c in range(NCORES)], axis=0
    )
    return np.ascontiguousarray(out, dtype=np.float32)
